# revision 5
# baseline (speedup 1.0000x reference)
"""Trainium2 Bass kernel: pairwise cosine similarity (nn_DistanceNetwork).

  target [4096, 1024] f32, ss [4096, 1024] f32
  out[i, j] = <target_i, ss_j> / max(||target_i|| * ||ss_j||, 1e-8)

Sharding: 8 NeuronCores as a 4x2 grid — 4 blocks of 1024 target rows x
2 blocks of 2048 ss rows. Each core computes its [1024, 2048] output block
locally; no collectives.

All data movement/layout runs on the host so the device kernel is a pure
GEMM: rows are L2-normalized (making the eps clamp dead and the GEMM the
full cosine matrix), transposed to [d, row] contraction-major layout, and
cast to bf16 (6 MB in / 8 MB out per core; no PE transposes or casts).

The fill is HBM-bandwidth-bound (~360 GB/s per-core share), so the load
schedule minimizes the bytes gating the first psum group: the m=0 column
slices of tT (0.25 MB) and the first s half (2 MB) land first, split
across the two HWDGE rings (Sync + Scalar); the rest of tT and the second
s half follow in consumption order. Groups run s-col-pair-outer so the
first 8 groups only touch the first s half. Everything else:
  - 16 psum groups, [128, 1024] 2-bank tiles, 8 k-chunk accumulation,
    bf16 matmuls stream 1 col/cycle (216 ns/MM warm); psum pool bufs=4
    (all 8 banks) so group handoffs never wait
  - no warmup: the DMA-paced first group self-warms the HAM clock gate
  - PSUM->SBUF copies per 512-col bank on DVE; stores per bank on the
    Sync HWDGE ring (no SWDGE: avoids GpSimd's ~4.6us end-of-kernel
    drain and ~2us store completion latency)
"""

from contextlib import ExitStack

import ml_dtypes
import numpy as np

import concourse.tile as tile
from concourse import bacc, mybir
from concourse.bass_utils import run_bass_kernel_spmd

F32 = mybir.dt.float32
BF16 = mybir.dt.bfloat16

P = 128
NB_COLS = 512          # psum bank width in fp32

N_FULL = 4096          # target rows
M_FULL = 4096          # ss rows
D_FULL = 1024          # feature dim
RB, CB = 4, 2          # core grid: target-row blocks x ss-row blocks
TM = N_FULL // RB      # 1024 target rows per core
SM = M_FULL // CB      # 2048 ss rows per core
N_CORES = 8
KC = D_FULL // P       # contraction chunks (8)
MT = TM // P           # output row chunks (8)
NP = SM // (2 * NB_COLS)  # output col pairs (2)

BF16_NP = np.dtype(ml_dtypes.bfloat16)


def _build_nc():
    """Build the per-core Bass program. Same program runs on all 8 cores."""
    nc = bacc.Bacc("TRN2", target_bir_lowering=False, debug=False)

    t = nc.dram_tensor("t", [KC, P, TM], BF16, kind="ExternalInput").ap()
    s = nc.dram_tensor("s", [KC, P, SM], BF16, kind="ExternalInput").ap()
    o = nc.dram_tensor("o", [TM, SM], F32, kind="ExternalOutput").ap()

    with tile.TileContext(nc) as tc, ExitStack() as ctx:
        big_pool = ctx.enter_context(tc.tile_pool(name="big", bufs=1))
        out_pool = ctx.enter_context(tc.tile_pool(name="outs", bufs=4))
        ps_mm_pool = ctx.enter_context(
            tc.tile_pool(name="ps_mm", bufs=4, space="PSUM"))

        # persistent contraction-major operands
        tT = big_pool.tile([P, KC, TM], BF16)
        sT = big_pool.tile([P, KC, SM], BF16)

        HS = SM // 2  # 1024: one s column-pair (2 psum banks wide)

        def load_t(q, k, c0, c1):
            q.dma_start(tT[:, k, c0:c1], t[k][:, c0:c1])

        def load_s(q, k, c0, c1):
            q.dma_start(sT[:, k, c0:c1], s[k][:, c0:c1])

        # group-0-critical bytes first, split across both HWDGE rings:
        #   sync:   t m=0 cols (8 x 32KB), sA even k (4 x 256KB),
        #           t cols 128:512 (8 x 96KB), then all output stores
        #   scalar: sA odd k (4 x 256KB), t cols 512:1024 (8 x 128KB),
        #           sB (8 x 256KB)
        for k in range(KC):
            load_t(nc.sync, k, 0, P)
        for k in range(1, KC, 2):
            load_s(nc.scalar, k, 0, HS)
        for k in range(0, KC, 2):
            load_s(nc.sync, k, 0, HS)
        for k in range(KC):
            load_t(nc.scalar, k, 4 * P, TM)
        for k in range(KC):
            load_t(nc.sync, k, P, 4 * P)
        for k in range(KC):
            load_s(nc.scalar, k, HS, SM)

        # main sweep, col-pair outer: psum group (np_, m) accumulates 8
        # k-chunks into a 2-bank [128, 1024] tile
        for np_ in range(NP):
            for m in range(MT):
                c0 = np_ * 2 * NB_COLS
                ps = ps_mm_pool.tile([P, 2 * NB_COLS], F32, tag="ps_mm",
                                     name=f"mps{np_}_{m}")
                for k in range(KC):
                    lhsT = tT[:, k, m * P:(m + 1) * P]
                    for j in range(2):
                        nc.tensor.matmul(
                            ps[:, j * NB_COLS:(j + 1) * NB_COLS],
                            lhsT,
                            sT[:, k, c0 + j * NB_COLS:c0 + (j + 1) * NB_COLS],
                            start=(k == 0),
                            stop=(k == KC - 1))
                o_s = out_pool.tile([P, 2 * NB_COLS], F32, tag="o_s",
                                    name=f"os{np_}_{m}")
                for j in range(2):
                    sl = slice(j * NB_COLS, (j + 1) * NB_COLS)
                    nc.vector.tensor_copy(o_s[:, sl], ps[:, sl])
                    nc.sync.dma_start(
                        o[m * P:(m + 1) * P,
                          c0 + j * NB_COLS:c0 + (j + 1) * NB_COLS],
                        o_s[:, sl])

    nc.compile()
    return nc


_NC_CACHE = None


def _get_nc():
    global _NC_CACHE
    if _NC_CACHE is None:
        _NC_CACHE = _build_nc()
    return _NC_CACHE


def _prep(block):
    """L2-normalize rows, transpose to [d, row] k-chunk layout, cast bf16."""
    n = np.linalg.norm(block, axis=1, keepdims=True)
    np.maximum(n, 1e-30, out=n)
    normed = block / n
    return np.ascontiguousarray(
        normed.T.reshape(KC, P, block.shape[0])).astype(BF16_NP)


def make_in_maps(target, ss):
    """Host prep: shard 4x2, normalize+transpose+cast each core's blocks."""
    t_blocks = [_prep(target[mb * TM:(mb + 1) * TM]) for mb in range(RB)]
    s_blocks = [_prep(ss[cb * SM:(cb + 1) * SM]) for cb in range(CB)]
    in_maps = []
    for c in range(N_CORES):
        mb, cb = divmod(c, CB)
        in_maps.append({"t": t_blocks[mb], "s": s_blocks[cb]})
    return in_maps


def kernel(target, ss):
    """Full cosine-similarity matrix on 8 NeuronCores; returns [4096, 4096] f32."""
    target = np.ascontiguousarray(np.asarray(target, dtype=np.float32))
    ss = np.ascontiguousarray(np.asarray(ss, dtype=np.float32))
    assert target.shape == (N_FULL, D_FULL) and ss.shape == (M_FULL, D_FULL)

    nc = _get_nc()
    in_maps = make_in_maps(target, ss)

    res = run_bass_kernel_spmd(nc, in_maps, list(range(N_CORES)))

    out = np.empty((N_FULL, M_FULL), dtype=np.float32)
    for c in range(N_CORES):
        mb, cb = divmod(c, CB)
        out[mb * TM:(mb + 1) * TM, cb * SM:(cb + 1) * SM] = \
            res.results[c]["o"]
    return out


# revision 6
# speedup vs baseline: 1.1440x; 1.1440x over previous
"""Trainium2 Bass kernel: pairwise cosine similarity (nn_DistanceNetwork).

  target [4096, 1024] f32, ss [4096, 1024] f32
  out[i, j] = <target_i, ss_j> / max(||target_i|| * ||ss_j||, 1e-8)

Sharding: 8 NeuronCores as a 4x2 grid — 4 blocks of 1024 target rows x
2 blocks of 2048 ss rows. Each core computes its [1024, 2048] output block
locally; no collectives.

All data movement/layout runs on the host so the device kernel is a pure
GEMM: rows are L2-normalized (making the eps clamp dead and the GEMM the
full cosine matrix), transposed to [d, row] contraction-major layout, and
cast to bf16 (6 MB in / 8 MB out per core; no PE transposes or casts).

The fill is HBM-bandwidth-bound (~360 GB/s per-core share), so the load
schedule minimizes the bytes gating the first psum group: the m=0 column
slices of tT (0.25 MB) and the first s half (2 MB) land first, split
across the two HWDGE rings (Sync + Scalar); the rest of tT and the second
s half follow in consumption order. Groups run s-col-pair-outer so the
first 8 groups only touch the first s half. Everything else:
  - 16 psum groups, [128, 1024] 2-bank tiles, 8 k-chunk accumulation,
    bf16 matmuls stream 1 col/cycle (216 ns/MM warm); psum pool bufs=4
    (all 8 banks) so group handoffs never wait
  - no warmup: the DMA-paced first group self-warms the HAM clock gate
  - PSUM->SBUF copies per 512-col bank on DVE; stores per bank on the
    Sync HWDGE ring (no SWDGE: avoids GpSimd's ~4.6us end-of-kernel
    drain and ~2us store completion latency)
"""

from contextlib import ExitStack

import ml_dtypes
import numpy as np

import concourse.tile as tile
from concourse import bacc, mybir
from concourse.bass_utils import run_bass_kernel_spmd

F32 = mybir.dt.float32
BF16 = mybir.dt.bfloat16

P = 128
NB_COLS = 512          # psum bank width in fp32

N_FULL = 4096          # target rows
M_FULL = 4096          # ss rows
D_FULL = 1024          # feature dim
RB, CB = 4, 2          # core grid: target-row blocks x ss-row blocks
TM = N_FULL // RB      # 1024 target rows per core
SM = M_FULL // CB      # 2048 ss rows per core
N_CORES = 8
KC = D_FULL // P       # contraction chunks (8)
MT = TM // P           # output row chunks (8)
NP = SM // (2 * NB_COLS)  # output col pairs (2)

BF16_NP = np.dtype(ml_dtypes.bfloat16)


def _build_nc():
    """Build the per-core Bass program. Same program runs on all 8 cores."""
    nc = bacc.Bacc("TRN2", target_bir_lowering=False, debug=False)

    t = nc.dram_tensor("t", [KC, P, TM], BF16, kind="ExternalInput").ap()
    s = nc.dram_tensor("s", [KC, P, SM], BF16, kind="ExternalInput").ap()
    o = nc.dram_tensor("o", [TM, SM], F32, kind="ExternalOutput").ap()

    with tile.TileContext(nc) as tc, ExitStack() as ctx:
        big_pool = ctx.enter_context(tc.tile_pool(name="big", bufs=1))
        out_pool = ctx.enter_context(tc.tile_pool(name="outs", bufs=4))
        ps_mm_pool = ctx.enter_context(
            tc.tile_pool(name="ps_mm", bufs=4, space="PSUM"))

        # persistent contraction-major operands
        tT = big_pool.tile([P, KC, TM], BF16)
        sT = big_pool.tile([P, KC, SM], BF16)

        HS = SM // 2  # 1024: one s column-pair (2 psum banks wide)

        def load_t(q, k, c0, c1):
            q.dma_start(tT[:, k, c0:c1], t[k][:, c0:c1])

        def load_s(q, k, c0, c1):
            q.dma_start(sT[:, k, c0:c1], s[k][:, c0:c1])

        # group-0-critical bytes (t cols 0:512 for m0-3 + the first s half)
        # first, interleaved by k across both HWDGE rings so pairs land in
        # consumption order; then t cols 512:1024, then the second s half.
        # All chunks stay >= 128KB (small strided loads serialize a ring).
        for k in range(KC):
            if k % 2 == 0:
                load_t(nc.sync, k, 0, 4 * P)
                load_s(nc.scalar, k, 0, HS)
            else:
                load_t(nc.scalar, k, 0, 4 * P)
                load_s(nc.sync, k, 0, HS)
        for k in range(KC):
            load_t(nc.sync, k, 4 * P, TM)
        for k in range(KC):
            load_s(nc.scalar, k, HS, SM)

        # main sweep, col-pair outer: psum group (np_, m) accumulates 8
        # k-chunks into a 2-bank [128, 1024] tile
        for np_ in range(NP):
            for m in range(MT):
                c0 = np_ * 2 * NB_COLS
                ps = ps_mm_pool.tile([P, 2 * NB_COLS], F32, tag="ps_mm",
                                     name=f"mps{np_}_{m}")
                for k in range(KC):
                    lhsT = tT[:, k, m * P:(m + 1) * P]
                    for j in range(2):
                        nc.tensor.matmul(
                            ps[:, j * NB_COLS:(j + 1) * NB_COLS],
                            lhsT,
                            sT[:, k, c0 + j * NB_COLS:c0 + (j + 1) * NB_COLS],
                            start=(k == 0),
                            stop=(k == KC - 1))
                o_s = out_pool.tile([P, 2 * NB_COLS], F32, tag="o_s",
                                    name=f"os{np_}_{m}")
                for j in range(2):
                    sl = slice(j * NB_COLS, (j + 1) * NB_COLS)
                    nc.vector.tensor_copy(o_s[:, sl], ps[:, sl])
                    nc.sync.dma_start(
                        o[m * P:(m + 1) * P,
                          c0 + j * NB_COLS:c0 + (j + 1) * NB_COLS],
                        o_s[:, sl])

    nc.compile()
    return nc


_NC_CACHE = None


def _get_nc():
    global _NC_CACHE
    if _NC_CACHE is None:
        _NC_CACHE = _build_nc()
    return _NC_CACHE


def _prep(block):
    """L2-normalize rows, transpose to [d, row] k-chunk layout, cast bf16."""
    n = np.linalg.norm(block, axis=1, keepdims=True)
    np.maximum(n, 1e-30, out=n)
    normed = block / n
    return np.ascontiguousarray(
        normed.T.reshape(KC, P, block.shape[0])).astype(BF16_NP)


def make_in_maps(target, ss):
    """Host prep: shard 4x2, normalize+transpose+cast each core's blocks."""
    t_blocks = [_prep(target[mb * TM:(mb + 1) * TM]) for mb in range(RB)]
    s_blocks = [_prep(ss[cb * SM:(cb + 1) * SM]) for cb in range(CB)]
    in_maps = []
    for c in range(N_CORES):
        mb, cb = divmod(c, CB)
        in_maps.append({"t": t_blocks[mb], "s": s_blocks[cb]})
    return in_maps


def kernel(target, ss):
    """Full cosine-similarity matrix on 8 NeuronCores; returns [4096, 4096] f32."""
    target = np.ascontiguousarray(np.asarray(target, dtype=np.float32))
    ss = np.ascontiguousarray(np.asarray(ss, dtype=np.float32))
    assert target.shape == (N_FULL, D_FULL) and ss.shape == (M_FULL, D_FULL)

    nc = _get_nc()
    in_maps = make_in_maps(target, ss)

    res = run_bass_kernel_spmd(nc, in_maps, list(range(N_CORES)))

    out = np.empty((N_FULL, M_FULL), dtype=np.float32)
    for c in range(N_CORES):
        mb, cb = divmod(c, CB)
        out[mb * TM:(mb + 1) * TM, cb * SM:(cb + 1) * SM] = \
            res.results[c]["o"]
    return out


# revision 8
# speedup vs baseline: 1.1460x; 1.0018x over previous
"""Trainium2 Bass kernel: pairwise cosine similarity (nn_DistanceNetwork).

  target [4096, 1024] f32, ss [4096, 1024] f32
  out[i, j] = <target_i, ss_j> / max(||target_i|| * ||ss_j||, 1e-8)

Sharding: 8 NeuronCores as a 4x2 grid — 4 blocks of 1024 target rows x
2 blocks of 2048 ss rows. Each core computes its [1024, 2048] output block
locally; no collectives.

All data movement/layout runs on the host so the device kernel is a pure
GEMM: rows are L2-normalized (making the eps clamp dead and the GEMM the
full cosine matrix), transposed to [d, row] contraction-major layout, and
cast to bf16 (6 MB in / 8 MB out per core; no PE transposes or casts).

The fill is HBM-bandwidth-bound (~360 GB/s per-core share), so the load
schedule minimizes the bytes gating the first psum group: the m=0 column
slices of tT (0.25 MB) and the first s half (2 MB) land first, split
across the two HWDGE rings (Sync + Scalar); the rest of tT and the second
s half follow in consumption order. Groups run s-col-pair-outer so the
first 8 groups only touch the first s half. Everything else:
  - 16 psum groups, [128, 1024] 2-bank tiles, 8 k-chunk accumulation,
    bf16 matmuls stream 1 col/cycle (216 ns/MM warm); psum pool bufs=4
    (all 8 banks) so group handoffs never wait
  - no warmup: the DMA-paced first group self-warms the HAM clock gate
  - PSUM->SBUF copies per 512-col bank on DVE; stores per bank on the
    Sync HWDGE ring (no SWDGE: avoids GpSimd's ~4.6us end-of-kernel
    drain and ~2us store completion latency)
"""

from contextlib import ExitStack

import ml_dtypes
import numpy as np

import concourse.tile as tile
from concourse import bacc, mybir
from concourse.bass_utils import run_bass_kernel_spmd

F32 = mybir.dt.float32
BF16 = mybir.dt.bfloat16

P = 128
NB_COLS = 512          # psum bank width in fp32

N_FULL = 4096          # target rows
M_FULL = 4096          # ss rows
D_FULL = 1024          # feature dim
RB, CB = 4, 2          # core grid: target-row blocks x ss-row blocks
TM = N_FULL // RB      # 1024 target rows per core
SM = M_FULL // CB      # 2048 ss rows per core
N_CORES = 8
KC = D_FULL // P       # contraction chunks (8)
MT = TM // P           # output row chunks (8)
NP = SM // (2 * NB_COLS)  # output col pairs (2)

BF16_NP = np.dtype(ml_dtypes.bfloat16)


def _build_nc():
    """Build the per-core Bass program. Same program runs on all 8 cores."""
    nc = bacc.Bacc("TRN2", target_bir_lowering=False, debug=False)

    t = nc.dram_tensor("t", [KC, P, TM], BF16, kind="ExternalInput").ap()
    s = nc.dram_tensor("s", [KC, P, SM], BF16, kind="ExternalInput").ap()
    o = nc.dram_tensor("o", [TM, SM], F32, kind="ExternalOutput").ap()

    with tile.TileContext(nc) as tc, ExitStack() as ctx:
        big_pool = ctx.enter_context(tc.tile_pool(name="big", bufs=1))
        out_pool = ctx.enter_context(tc.tile_pool(name="outs", bufs=4))
        ps_mm_pool = ctx.enter_context(
            tc.tile_pool(name="ps_mm", bufs=4, space="PSUM"))

        # persistent contraction-major operands
        tT = big_pool.tile([P, KC, TM], BF16)
        sT = big_pool.tile([P, KC, SM], BF16)

        HS = SM // 2  # 1024: one s column-pair (2 psum banks wide)

        def load_t(q, k, c0, c1):
            q.dma_start(tT[:, k, c0:c1], t[k][:, c0:c1])

        def load_s(q, k, c0, c1):
            q.dma_start(sT[:, k, c0:c1], s[k][:, c0:c1])

        # group-0-critical bytes (t cols 0:512 for m0-3 + the first s half)
        # first, interleaved by k across both HWDGE rings so pairs land in
        # consumption order; then t cols 512:1024, then the second s half.
        # All chunks stay >= 128KB (small strided loads serialize a ring).
        for k in range(KC):
            if k % 2 == 0:
                load_t(nc.sync, k, 0, 4 * P)
                load_s(nc.scalar, k, 0, HS)
            else:
                load_t(nc.scalar, k, 0, 4 * P)
                load_s(nc.sync, k, 0, HS)
        for k in range(KC):
            load_t(nc.sync, k, 4 * P, TM)
        for k in range(KC):
            load_s(nc.scalar, k, HS, SM)

        def evac(ps, m, np_):
            """Copy psum group (np_, m) to SBUF per bank and store it."""
            c0 = np_ * 2 * NB_COLS
            o_s = out_pool.tile([P, 2 * NB_COLS], F32, tag="o_s",
                                name=f"os{np_}_{m}")
            for j in range(2):
                sl = slice(j * NB_COLS, (j + 1) * NB_COLS)
                nc.vector.tensor_copy(o_s[:, sl], ps[:, sl])
                nc.sync.dma_start(
                    o[m * P:(m + 1) * P,
                      c0 + j * NB_COLS:c0 + (j + 1) * NB_COLS],
                    o_s[:, sl])

        # ~6 throwaway matmuls that depend only on the first t chunk: keep
        # the PE busy (and the HAM clock gate warming) while the first s
        # chunk is still in flight
        warm = ps_mm_pool.tile([P, 2 * NB_COLS], F32, tag="ps_mm",
                               name="warm")
        for w in range(6):
            nc.tensor.matmul(warm[:, 0:NB_COLS], tT[:, 0, 0:P],
                             tT[:, 0, 0:NB_COLS], start=True, stop=True)

        # phase 1 — fill-overlapped quad: groups (np=0, m=0..3) advance
        # k-chunk by k-chunk together (4 tiles = 8 psum banks), so each
        # arriving (t,sA) chunk pair immediately feeds 8 real matmuls and
        # the HBM-paced fill is covered by useful PE work
        qs = [ps_mm_pool.tile([P, 2 * NB_COLS], F32, tag="ps_mm",
                              name=f"q{m}") for m in range(4)]
        for k in range(KC):
            for m in range(4):
                lhsT = tT[:, k, m * P:(m + 1) * P]
                for j in range(2):
                    nc.tensor.matmul(
                        qs[m][:, j * NB_COLS:(j + 1) * NB_COLS],
                        lhsT,
                        sT[:, k, j * NB_COLS:(j + 1) * NB_COLS],
                        start=(k == 0),
                        stop=(k == KC - 1))
            if k == KC - 1:
                # evacuate each quad group as soon as its last matmul
                # retires so the next serial group's psum frees up in time
                for m in range(4):
                    evac(qs[m], m, 0)

        # phase 2 — serial groups: (np0, m4..7) then (np1, m0..7)
        serial = [(0, m) for m in range(4, MT)] + \
                 [(1, m) for m in range(MT)]
        for np_, m in serial:
            c0 = np_ * 2 * NB_COLS
            ps = ps_mm_pool.tile([P, 2 * NB_COLS], F32, tag="ps_mm",
                                 name=f"mps{np_}_{m}")
            last = (np_, m) == serial[-1]
            if last:
                # bank-split k loops: bank A's copy+store fully overlap
                # bank B's matmuls, shortening the end-of-kernel tail
                for j in range(2):
                    for k in range(KC):
                        nc.tensor.matmul(
                            ps[:, j * NB_COLS:(j + 1) * NB_COLS],
                            tT[:, k, m * P:(m + 1) * P],
                            sT[:, k, c0 + j * NB_COLS:c0 + (j + 1) * NB_COLS],
                            start=(k == 0),
                            stop=(k == KC - 1))
            else:
                for k in range(KC):
                    lhsT = tT[:, k, m * P:(m + 1) * P]
                    for j in range(2):
                        nc.tensor.matmul(
                            ps[:, j * NB_COLS:(j + 1) * NB_COLS],
                            lhsT,
                            sT[:, k, c0 + j * NB_COLS:c0 + (j + 1) * NB_COLS],
                            start=(k == 0),
                            stop=(k == KC - 1))
            evac(ps, m, np_)

    nc.compile()
    return nc


_NC_CACHE = None


def _get_nc():
    global _NC_CACHE
    if _NC_CACHE is None:
        _NC_CACHE = _build_nc()
    return _NC_CACHE


def _prep(block):
    """L2-normalize rows, transpose to [d, row] k-chunk layout, cast bf16."""
    n = np.linalg.norm(block, axis=1, keepdims=True)
    np.maximum(n, 1e-30, out=n)
    normed = block / n
    return np.ascontiguousarray(
        normed.T.reshape(KC, P, block.shape[0])).astype(BF16_NP)


def make_in_maps(target, ss):
    """Host prep: shard 4x2, normalize+transpose+cast each core's blocks."""
    t_blocks = [_prep(target[mb * TM:(mb + 1) * TM]) for mb in range(RB)]
    s_blocks = [_prep(ss[cb * SM:(cb + 1) * SM]) for cb in range(CB)]
    in_maps = []
    for c in range(N_CORES):
        mb, cb = divmod(c, CB)
        in_maps.append({"t": t_blocks[mb], "s": s_blocks[cb]})
    return in_maps


def kernel(target, ss):
    """Full cosine-similarity matrix on 8 NeuronCores; returns [4096, 4096] f32."""
    target = np.ascontiguousarray(np.asarray(target, dtype=np.float32))
    ss = np.ascontiguousarray(np.asarray(ss, dtype=np.float32))
    assert target.shape == (N_FULL, D_FULL) and ss.shape == (M_FULL, D_FULL)

    nc = _get_nc()
    in_maps = make_in_maps(target, ss)

    res = run_bass_kernel_spmd(nc, in_maps, list(range(N_CORES)))

    out = np.empty((N_FULL, M_FULL), dtype=np.float32)
    for c in range(N_CORES):
        mb, cb = divmod(c, CB)
        out[mb * TM:(mb + 1) * TM, cb * SM:(cb + 1) * SM] = \
            res.results[c]["o"]
    return out


# revision 14
# speedup vs baseline: 1.1632x; 1.0150x over previous
"""Trainium2 Bass kernel: pairwise cosine similarity (nn_DistanceNetwork).

  target [4096, 1024] f32, ss [4096, 1024] f32
  out[i, j] = <target_i, ss_j> / max(||target_i|| * ||ss_j||, 1e-8)

Sharding: 8 NeuronCores as a 4x2 grid — 4 blocks of 1024 target rows x
2 blocks of 2048 ss rows. Each core computes its [1024, 2048] output block
locally; no collectives.

All data movement/layout runs on the host so the device kernel is a pure
GEMM: rows are L2-normalized (making the eps clamp dead and the GEMM the
full cosine matrix), transposed to [d, row] contraction-major layout, and
cast to bf16 (6 MB in / 8 MB out per core; no PE transposes or casts).

The fill is HBM-bandwidth-bound (~360 GB/s per-core share), so the load
schedule minimizes the bytes gating the first psum group: the m=0 column
slices of tT (0.25 MB) and the first s half (2 MB) land first, split
across the two HWDGE rings (Sync + Scalar); the rest of tT and the second
s half follow in consumption order. Groups run s-col-pair-outer so the
first 8 groups only touch the first s half. Everything else:
  - 16 psum groups, [128, 1024] 2-bank tiles, 8 k-chunk accumulation,
    bf16 matmuls stream 1 col/cycle (216 ns/MM warm); psum pool bufs=4
    (all 8 banks) so group handoffs never wait
  - no warmup: the DMA-paced first group self-warms the HAM clock gate
  - PSUM->SBUF copies per 512-col bank on DVE; stores per bank on the
    Sync HWDGE ring (no SWDGE: avoids GpSimd's ~4.6us end-of-kernel
    drain and ~2us store completion latency)
"""

from contextlib import ExitStack

import ml_dtypes
import numpy as np

import concourse.tile as tile
from concourse import bacc, mybir
from concourse.bass_utils import run_bass_kernel_spmd

F32 = mybir.dt.float32
BF16 = mybir.dt.bfloat16

P = 128
NB_COLS = 512          # psum bank width in fp32

N_FULL = 4096          # target rows
M_FULL = 4096          # ss rows
D_FULL = 1024          # feature dim
RB, CB = 4, 2          # core grid: target-row blocks x ss-row blocks
TM = N_FULL // RB      # 1024 target rows per core
SM = M_FULL // CB      # 2048 ss rows per core
N_CORES = 8
KC = D_FULL // P       # contraction chunks (8)
MT = TM // P           # output row chunks (8)
NP = SM // (2 * NB_COLS)  # output col pairs (2)

BF16_NP = np.dtype(ml_dtypes.bfloat16)


def _build_nc():
    """Build the per-core Bass program. Same program runs on all 8 cores."""
    nc = bacc.Bacc("TRN2", target_bir_lowering=False, debug=False)

    t = nc.dram_tensor("t", [KC, P, TM], BF16, kind="ExternalInput").ap()
    s = nc.dram_tensor("s", [KC, P, SM], BF16, kind="ExternalInput").ap()
    o = nc.dram_tensor("o", [TM, SM], F32, kind="ExternalOutput").ap()

    with tile.TileContext(nc) as tc, ExitStack() as ctx:
        big_pool = ctx.enter_context(tc.tile_pool(name="big", bufs=1))
        out_pool = ctx.enter_context(tc.tile_pool(name="outs", bufs=6))
        ps_mm_pool = ctx.enter_context(
            tc.tile_pool(name="ps_mm", bufs=4, space="PSUM"))

        # persistent contraction-major operands
        tT = big_pool.tile([P, KC, TM], BF16)
        sT = big_pool.tile([P, KC, SM], BF16)

        HS = SM // 2  # 1024: one s column-pair (2 psum banks wide)

        def load_t(q, k, c0, c1):
            q.dma_start(tT[:, k, c0:c1], t[k][:, c0:c1])

        def load_s(q, k, c0, c1):
            q.dma_start(sT[:, k, c0:c1], s[k][:, c0:c1])

        # group-0-critical bytes (t cols 0:512 for m0-3 + the first s half)
        # first, interleaved by k across both HWDGE rings so pairs land in
        # consumption order; then t cols 512:1024, then the second s half.
        # Chunks stay >= 128KB (small strided loads serialize a ring),
        # except the very first s chunk which is split so the first real
        # matmul's inputs surface sooner out of the 8-core startup herd.
        for k in range(KC):
            if k % 2 == 0:
                load_t(nc.sync, k, 0, 4 * P)
                if k == 0:
                    load_s(nc.scalar, k, 0, NB_COLS)
                    load_s(nc.scalar, k, NB_COLS, HS)
                else:
                    load_s(nc.scalar, k, 0, HS)
            else:
                load_t(nc.scalar, k, 0, 4 * P)
                load_s(nc.sync, k, 0, HS)
        for k in range(KC):
            load_t(nc.sync, k, 4 * P, TM)
        for k in range(KC):
            load_s(nc.scalar, k, HS, SM)

        def evac(ps, m, np_):
            """Copy psum group (np_, m) to SBUF per bank and store it."""
            c0 = np_ * 2 * NB_COLS
            o_s = out_pool.tile([P, 2 * NB_COLS], F32, tag="o_s",
                                name=f"os{np_}_{m}")
            for j in range(2):
                sl = slice(j * NB_COLS, (j + 1) * NB_COLS)
                nc.vector.tensor_copy(o_s[:, sl], ps[:, sl])
                nc.sync.dma_start(
                    o[m * P:(m + 1) * P,
                      c0 + j * NB_COLS:c0 + (j + 1) * NB_COLS],
                    o_s[:, sl])

        # throwaway matmuls on a memset tile — no DMA dependency at all, so
        # the PE starts (and the HAM clock gate warms to 2.4 GHz) during
        # the ~5us the first input loads spend in the 8-core startup herd
        warm_in = big_pool.tile([P, NB_COLS], BF16)
        nc.gpsimd.memset(warm_in[:], 0.0)
        warm = ps_mm_pool.tile([P, 2 * NB_COLS], F32, tag="ps_mm",
                               name="warm")
        for w in range(16):
            nc.tensor.matmul(warm[:, 0:NB_COLS], warm_in[:, 0:P],
                             warm_in[:], start=True, stop=True)

        # phase 1 — fill-overlapped quad: groups (np=0, m=0..3) advance
        # k-chunk by k-chunk together (4 tiles = 8 psum banks), so each
        # arriving (t,sA) chunk pair immediately feeds 8 real matmuls and
        # the HBM-paced fill is covered by useful PE work
        qs = [ps_mm_pool.tile([P, 2 * NB_COLS], F32, tag="ps_mm",
                              name=f"q{m}") for m in range(4)]
        for k in range(KC):
            for m in range(4):
                lhsT = tT[:, k, m * P:(m + 1) * P]
                for j in range(2):
                    nc.tensor.matmul(
                        qs[m][:, j * NB_COLS:(j + 1) * NB_COLS],
                        lhsT,
                        sT[:, k, j * NB_COLS:(j + 1) * NB_COLS],
                        start=(k == 0),
                        stop=(k == KC - 1))
                if k == KC - 1:
                    # evacuate each quad group as soon as its last matmul
                    # retires: DVE drains q0 while the PE finishes q1-q3,
                    # so the first serial group's psum frees up in time
                    evac(qs[m], m, 0)

        # phase 2 — serial groups: (np0, m4..7) then (np1, m0..7)
        serial = [(0, m) for m in range(4, MT)] + \
                 [(1, m) for m in range(MT)]
        for np_, m in serial:
            c0 = np_ * 2 * NB_COLS
            ps = ps_mm_pool.tile([P, 2 * NB_COLS], F32, tag="ps_mm",
                                 name=f"mps{np_}_{m}")
            last = (np_, m) == serial[-1]
            if last:
                # bank-split k loops: bank A's copy+store fully overlap
                # bank B's matmuls; bank B evacuates in 256-col quarters
                # so the final (smallest) store leaves earliest and its
                # HBM completion doesn't stretch the epilogue
                for j in range(2):
                    for k in range(KC):
                        nc.tensor.matmul(
                            ps[:, j * NB_COLS:(j + 1) * NB_COLS],
                            tT[:, k, m * P:(m + 1) * P],
                            sT[:, k, c0 + j * NB_COLS:c0 + (j + 1) * NB_COLS],
                            start=(k == 0),
                            stop=(k == KC - 1))
                o_s = out_pool.tile([P, 2 * NB_COLS], F32, tag="o_s",
                                    name="os_last")
                q = NB_COLS // 2
                for piece in range(4):
                    sl = slice(piece * q, (piece + 1) * q)
                    nc.vector.tensor_copy(o_s[:, sl], ps[:, sl])
                    nc.sync.dma_start(
                        o[m * P:(m + 1) * P,
                          c0 + piece * q:c0 + (piece + 1) * q],
                        o_s[:, sl])
                continue
            else:
                for k in range(KC):
                    lhsT = tT[:, k, m * P:(m + 1) * P]
                    for j in range(2):
                        nc.tensor.matmul(
                            ps[:, j * NB_COLS:(j + 1) * NB_COLS],
                            lhsT,
                            sT[:, k, c0 + j * NB_COLS:c0 + (j + 1) * NB_COLS],
                            start=(k == 0),
                            stop=(k == KC - 1))
            evac(ps, m, np_)

    nc.compile()
    return nc


_NC_CACHE = None


def _get_nc():
    global _NC_CACHE
    if _NC_CACHE is None:
        _NC_CACHE = _build_nc()
    return _NC_CACHE


def _prep(block):
    """L2-normalize rows, transpose to [d, row] k-chunk layout, cast bf16."""
    n = np.linalg.norm(block, axis=1, keepdims=True)
    np.maximum(n, 1e-30, out=n)
    normed = block / n
    return np.ascontiguousarray(
        normed.T.reshape(KC, P, block.shape[0])).astype(BF16_NP)


def make_in_maps(target, ss):
    """Host prep: shard 4x2, normalize+transpose+cast each core's blocks."""
    t_blocks = [_prep(target[mb * TM:(mb + 1) * TM]) for mb in range(RB)]
    s_blocks = [_prep(ss[cb * SM:(cb + 1) * SM]) for cb in range(CB)]
    in_maps = []
    for c in range(N_CORES):
        mb, cb = divmod(c, CB)
        in_maps.append({"t": t_blocks[mb], "s": s_blocks[cb]})
    return in_maps


def kernel(target, ss):
    """Full cosine-similarity matrix on 8 NeuronCores; returns [4096, 4096] f32."""
    target = np.ascontiguousarray(np.asarray(target, dtype=np.float32))
    ss = np.ascontiguousarray(np.asarray(ss, dtype=np.float32))
    assert target.shape == (N_FULL, D_FULL) and ss.shape == (M_FULL, D_FULL)

    nc = _get_nc()
    in_maps = make_in_maps(target, ss)

    res = run_bass_kernel_spmd(nc, in_maps, list(range(N_CORES)))

    out = np.empty((N_FULL, M_FULL), dtype=np.float32)
    for c in range(N_CORES):
        mb, cb = divmod(c, CB)
        out[mb * TM:(mb + 1) * TM, cb * SM:(cb + 1) * SM] = \
            res.results[c]["o"]
    return out


# revision 16
# speedup vs baseline: 1.1976x; 1.0296x over previous
"""Trainium2 Bass kernel: pairwise cosine similarity (nn_DistanceNetwork).

  target [4096, 1024] f32, ss [4096, 1024] f32
  out[i, j] = <target_i, ss_j> / max(||target_i|| * ||ss_j||, 1e-8)

Sharding: 8 NeuronCores as a 4x2 grid — 4 blocks of 1024 target rows x
2 blocks of 2048 ss rows. Each core computes its [1024, 2048] output block
locally; no collectives.

All data movement/layout runs on the host so the device kernel is a pure
GEMM: rows are L2-normalized (making the eps clamp dead and the GEMM the
full cosine matrix), transposed to [d, row] contraction-major layout, and
cast to bf16 (6 MB in / 8 MB out per core; no PE transposes or casts).

The fill is HBM-bandwidth-bound (~360 GB/s per-core share), so the load
schedule minimizes the bytes gating the first psum group: the m=0 column
slices of tT (0.25 MB) and the first s half (2 MB) land first, split
across the two HWDGE rings (Sync + Scalar); the rest of tT and the second
s half follow in consumption order. Groups run s-col-pair-outer so the
first 8 groups only touch the first s half. Everything else:
  - 16 psum groups, [128, 1024] 2-bank tiles, 8 k-chunk accumulation,
    bf16 matmuls stream 1 col/cycle (216 ns/MM warm); psum pool bufs=4
    (all 8 banks) so group handoffs never wait
  - no warmup: the DMA-paced first group self-warms the HAM clock gate
  - PSUM->SBUF copies per 512-col bank on DVE; stores per bank on the
    Sync HWDGE ring (no SWDGE: avoids GpSimd's ~4.6us end-of-kernel
    drain and ~2us store completion latency)
"""

from contextlib import ExitStack

import ml_dtypes
import numpy as np

import concourse.tile as tile
from concourse import bacc, mybir
from concourse.bass_utils import run_bass_kernel_spmd

F32 = mybir.dt.float32
BF16 = mybir.dt.bfloat16

P = 128
NB_COLS = 512          # psum bank width in fp32

N_FULL = 4096          # target rows
M_FULL = 4096          # ss rows
D_FULL = 1024          # feature dim
RB, CB = 4, 2          # core grid: target-row blocks x ss-row blocks
TM = N_FULL // RB      # 1024 target rows per core
SM = M_FULL // CB      # 2048 ss rows per core
N_CORES = 8
KC = D_FULL // P       # contraction chunks (8)
MT = TM // P           # output row chunks (8)
NP = SM // (2 * NB_COLS)  # output col pairs (2)

BF16_NP = np.dtype(ml_dtypes.bfloat16)


def _build_nc():
    """Build the per-core Bass program. Same program runs on all 8 cores."""
    nc = bacc.Bacc("TRN2", target_bir_lowering=False, debug=False)

    t = nc.dram_tensor("t", [KC, P, TM], BF16, kind="ExternalInput").ap()
    s = nc.dram_tensor("s", [KC, P, SM], BF16, kind="ExternalInput").ap()
    o = nc.dram_tensor("o", [TM, SM], F32, kind="ExternalOutput").ap()

    with tile.TileContext(nc) as tc, ExitStack() as ctx:
        big_pool = ctx.enter_context(tc.tile_pool(name="big", bufs=1))
        out_pool = ctx.enter_context(tc.tile_pool(name="outs", bufs=10))
        ps_mm_pool = ctx.enter_context(
            tc.tile_pool(name="ps_mm", bufs=8, space="PSUM"))

        # persistent contraction-major operands
        tT = big_pool.tile([P, KC, TM], BF16)
        sT = big_pool.tile([P, KC, SM], BF16)

        HS = SM // 2  # 1024: one s column-pair (2 psum banks wide)

        def load_t(q, k, c0, c1):
            q.dma_start(tT[:, k, c0:c1], t[k][:, c0:c1])

        def load_s(q, k, c0, c1):
            q.dma_start(sT[:, k, c0:c1], s[k][:, c0:c1])

        # group-0-critical bytes (t cols 0:512 for m0-3 + the first s half)
        # first, interleaved by k across both HWDGE rings so pairs land in
        # consumption order; then t cols 512:1024, then the second s half.
        # Chunks stay >= 128KB (small strided loads serialize a ring),
        # except the very first s chunk which is split so the first real
        # matmul's inputs surface sooner out of the 8-core startup herd.
        for k in range(KC):
            if k % 2 == 0:
                load_t(nc.sync, k, 0, 4 * P)
                if k == 0:
                    load_s(nc.scalar, k, 0, NB_COLS)
                    load_s(nc.scalar, k, NB_COLS, HS)
                else:
                    load_s(nc.scalar, k, 0, HS)
            else:
                load_t(nc.scalar, k, 0, 4 * P)
                load_s(nc.sync, k, 0, HS)
        for k in range(KC):
            load_t(nc.sync, k, 4 * P, TM)
        for k in range(KC):
            load_s(nc.scalar, k, HS, SM)

        def evac_bank(ps, m, np_, j):
            """Copy one psum bank to SBUF and store it. ps is a single-bank
            tile, so this only depends on that bank's own matmuls and can
            overlap the sibling bank's accumulation on the PE."""
            c0 = np_ * 2 * NB_COLS + j * NB_COLS
            o_s = out_pool.tile([P, NB_COLS], F32, tag="o_s",
                                name=f"os{np_}_{m}_{j}")
            nc.vector.tensor_copy(o_s[:], ps[:])
            nc.sync.dma_start(o[m * P:(m + 1) * P, c0:c0 + NB_COLS], o_s[:])

        # throwaway matmuls on a memset tile — no DMA dependency at all, so
        # the PE starts (and the HAM clock gate warms to 2.4 GHz) during
        # the ~6us the first input loads' completion semaphores spend in
        # the 8-core startup herd
        warm_in = big_pool.tile([P, NB_COLS], BF16)
        nc.gpsimd.memset(warm_in[:], 0.0)
        warm = ps_mm_pool.tile([P, NB_COLS], F32, tag="ps_mm", name="warm")
        for w in range(12):
            nc.tensor.matmul(warm[:], warm_in[:, 0:P], warm_in[:],
                             start=True, stop=True)

        # phase 1 — fill-overlapped quad: groups (np=0, m=0..3) advance
        # k-chunk by k-chunk together (8 single-bank tiles = all 8 psum
        # banks), so each arriving (t,sA) chunk pair immediately feeds 8
        # real matmuls and the HBM-paced fill is covered by useful PE work
        qt = [[ps_mm_pool.tile([P, NB_COLS], F32, tag="ps_mm",
                               name=f"q{m}_{j}") for j in range(2)]
              for m in range(4)]
        for k in range(KC):
            for m in range(4):
                lhsT = tT[:, k, m * P:(m + 1) * P]
                for j in range(2):
                    nc.tensor.matmul(
                        qt[m][j][:],
                        lhsT,
                        sT[:, k, j * NB_COLS:(j + 1) * NB_COLS],
                        start=(k == 0),
                        stop=(k == KC - 1))
                if k == KC - 1:
                    # evacuate each quad bank as soon as its last matmul
                    # retires: DVE drains q0 while the PE finishes q1-q3
                    evac_bank(qt[m][0], m, 0, 0)
                    evac_bank(qt[m][1], m, 0, 1)

        # phase 2 — serial groups: (np0, m4..7) then (np1, m0..7), each as
        # two bank-split k loops so bank A's copy+store fully overlap bank
        # B's matmuls (this is also what makes the end-of-kernel tail just
        # one copy + one store deep)
        serial = [(0, m) for m in range(4, MT)] + \
                 [(1, m) for m in range(MT)]
        for np_, m in serial:
            c0 = np_ * 2 * NB_COLS
            for j in range(2):
                ps = ps_mm_pool.tile([P, NB_COLS], F32, tag="ps_mm",
                                     name=f"mps{np_}_{m}_{j}")
                for k in range(KC):
                    nc.tensor.matmul(
                        ps[:],
                        tT[:, k, m * P:(m + 1) * P],
                        sT[:, k, c0 + j * NB_COLS:c0 + (j + 1) * NB_COLS],
                        start=(k == 0),
                        stop=(k == KC - 1))
                evac_bank(ps, m, np_, j)

    nc.compile()
    return nc


_NC_CACHE = None


def _get_nc():
    global _NC_CACHE
    if _NC_CACHE is None:
        _NC_CACHE = _build_nc()
    return _NC_CACHE


def _prep(block):
    """L2-normalize rows, transpose to [d, row] k-chunk layout, cast bf16."""
    n = np.linalg.norm(block, axis=1, keepdims=True)
    np.maximum(n, 1e-30, out=n)
    normed = block / n
    return np.ascontiguousarray(
        normed.T.reshape(KC, P, block.shape[0])).astype(BF16_NP)


def make_in_maps(target, ss):
    """Host prep: shard 4x2, normalize+transpose+cast each core's blocks."""
    t_blocks = [_prep(target[mb * TM:(mb + 1) * TM]) for mb in range(RB)]
    s_blocks = [_prep(ss[cb * SM:(cb + 1) * SM]) for cb in range(CB)]
    in_maps = []
    for c in range(N_CORES):
        mb, cb = divmod(c, CB)
        in_maps.append({"t": t_blocks[mb], "s": s_blocks[cb]})
    return in_maps


def kernel(target, ss):
    """Full cosine-similarity matrix on 8 NeuronCores; returns [4096, 4096] f32."""
    target = np.ascontiguousarray(np.asarray(target, dtype=np.float32))
    ss = np.ascontiguousarray(np.asarray(ss, dtype=np.float32))
    assert target.shape == (N_FULL, D_FULL) and ss.shape == (M_FULL, D_FULL)

    nc = _get_nc()
    in_maps = make_in_maps(target, ss)

    res = run_bass_kernel_spmd(nc, in_maps, list(range(N_CORES)))

    out = np.empty((N_FULL, M_FULL), dtype=np.float32)
    for c in range(N_CORES):
        mb, cb = divmod(c, CB)
        out[mb * TM:(mb + 1) * TM, cb * SM:(cb + 1) * SM] = \
            res.results[c]["o"]
    return out
